# revision 10
# baseline (speedup 1.0000x reference)
"""Trainium2 Bass kernel for nn_AttentionBlock64: batch of 8192 independent
64x64 attention tiles, data-parallel across 8 NeuronCores.

out[b] = (softmax(q[b] @ k[b]^T) @ v[b]) @ proj[b] + residual[b]

Math restructure (per tile, all accumulation in fp32 PSUM):
  S   = q @ k^T                    (fp16 inputs, PE)
  E   = exp(S)                     (ACT, fp32 in, bf16 out, accum_out -> rowsum r)
  E^T = transpose(E)               (PE transpose)
  U^T = mm(lhsT=v,   rhs=E^T)      (bf16)
  P   = mm(lhsT=U^T, rhs=proj)     (fp32 psum)
  out = P * (1/r) + residual       (ACT copy-scale; residual added by DMA accum)
The softmax denominator is deferred: row scaling commutes with the
right-multiplications by v and proj.

Layout: tiles processed in pairs stacked on partition halves; quads (4 tiles)
share one 128x128 PE transpose; matmuls are dispatched to distinct 64x64
array quadrants via tile_position so they run concurrently. Concurrent
row-groups must write distinct PSUM banks or distinct partitions (same-bank
same-partition concurrent PE writes wedge the device) - hence the A/B bank
split by pair parity for M1/M2.
"""

import numpy as np

import concourse.bass as bass
import concourse.bacc as bacc
import concourse.mybir as mybir
from concourse import tile

F32 = mybir.dt.float32
F16 = mybir.dt.float16
BF16 = mybir.dt.bfloat16

T = 64          # tile dim
N_CORES = 8
B_FULL = 8192
NT_CORE = B_FULL // N_CORES   # tiles per core


def build(nt=NT_CORE, sbp=8, m1_dt=F16, m23_dt=BF16, reps=1):
    """Build the SPMD single-core program processing `nt` tiles.

    sbp: pairs per superblock (superblock = 2*sbp tiles).
    reps: repeat the whole computation (for timing-by-slope benchmarks).
    """
    sbt = 2 * sbp            # tiles per superblock
    nq = sbp // 2            # quads per superblock
    assert nt % sbt == 0
    nsb = nt // sbt

    nc = bacc.Bacc("TRN2", target_bir_lowering=False, debug=False)

    q_d = nc.dram_tensor("q", [nt, T, T], F32, kind="ExternalInput").ap()
    k_d = nc.dram_tensor("k", [nt, T, T], F32, kind="ExternalInput").ap()
    v_d = nc.dram_tensor("v", [nt, T, T], F32, kind="ExternalInput").ap()
    p_d = nc.dram_tensor("proj", [nt, T, T], F32, kind="ExternalInput").ap()
    r_d = nc.dram_tensor("residual", [nt, T, T], F32, kind="ExternalInput").ap()
    o_d = nc.dram_tensor("out", [nt, T, T], F32, kind="ExternalOutput").ap()

    # pair view: partition = (h i) across the 2 tiles of a pair
    pairv = lambda ap: ap.rearrange("(ns pb h) i j -> ns (h i) pb j", pb=sbp, h=2)
    qv, kv, pv, rv, ov = pairv(q_d), pairv(k_d), pairv(p_d), pairv(r_d), pairv(o_d)
    # v block view: tile (4g+tl) with i on partitions (half chosen by slicing tl)
    vv = v_d.rearrange("(ns g tl) i j -> ns i g tl j", g=nq, tl=4)

    with tile.TileContext(nc) as tc:
        with (
            tc.tile_pool(name="consts", bufs=1) as consts,
            tc.tile_pool(name="qkin", bufs=3) as qkin,
            tc.tile_pool(name="vpin", bufs=3) as vpin,
            tc.tile_pool(name="vp16", bufs=2) as vp16,
            tc.tile_pool(name="mid", bufs=2) as mid,
            tc.tile_pool(name="outp", bufs=3) as outp,
            tc.tile_pool(name="ps_tqk", bufs=1, space="PSUM") as ps_tqk,
            tc.tile_pool(name="ps_sa", bufs=1, space="PSUM") as ps_sa,
            tc.tile_pool(name="ps_sb", bufs=1, space="PSUM") as ps_sb,
            tc.tile_pool(name="ps_te", bufs=1, space="PSUM") as ps_te,
            tc.tile_pool(name="ps_ua", bufs=1, space="PSUM") as ps_ua,
            tc.tile_pool(name="ps_ub", bufs=1, space="PSUM") as ps_ub,
            tc.tile_pool(name="ps_p", bufs=1, space="PSUM") as ps_p,
        ):
            # --- identity matrices for PE transposes ---
            idf = consts.tile([128, 128], F32, tag="idf")
            idbf = consts.tile([128, 128], m23_dt, tag="idbf")
            onesf = consts.tile([128, 128], F32, tag="onesf")
            onesbf = consts.tile([128, 128], m23_dt, tag="onesbf")
            nc.gpsimd.memset(onesf[:], 1.0)
            nc.gpsimd.memset(onesbf[:], 1.0)
            for src, dst in ((onesf, idf), (onesbf, idbf)):
                nc.gpsimd.affine_select(
                    out=dst[:], in_=src[:], pattern=[[-1, 128]],
                    compare_op=mybir.AluOpType.is_equal, fill=0.0,
                    base=0, channel_multiplier=1,
                )

            for s_rep in range(reps * nsb):
                s = s_rep % nsb
                # ---- loads (HWDGE, f32) ----
                ql = qkin.tile([128, sbp, T], F32, tag="ql")
                kl = qkin.tile([128, sbp, T], F32, tag="kl")
                vl = vpin.tile([128, nq, 2, T], F32, tag="vl")
                pl = vpin.tile([128, sbp, T], F32, tag="pl")
                nc.sync.dma_start(out=ql[:], in_=qv[s])
                nc.sync.dma_start(out=kl[:], in_=kv[s])
                for tl in range(4):
                    half = slice(0, 64) if tl < 2 else slice(64, 128)
                    nc.sync.dma_start(out=vl[half, :, tl % 2, :],
                                      in_=vv[s, :, :, tl, :])
                nc.sync.dma_start(out=pl[:], in_=pv[s])

                # ---- cast v/proj to bf16 on gpsimd (otherwise idle) ----
                vl16 = vp16.tile([128, nq, 2, T], m23_dt, tag="vl16")
                pl16 = vp16.tile([128, sbp, T], m23_dt, tag="pl16")
                nc.gpsimd.tensor_copy(vl16[:], vl[:])
                nc.gpsimd.tensor_copy(pl16[:], pl[:])

                # ---- transpose q,k quads on PE (f32) ----
                tqk = ps_tqk.tile([128, nq, 2, 128], F32, tag="tqk")
                # f32 quads: 1KB per quad-slot, zero region = 2KB -> flags
                # restart per pair of quads
                for g in range(nq):
                    nc.tensor.matmul(
                        tqk[:, g, 0, :], ql[:, 2 * g : 2 * g + 2, :], idf[:],
                        is_transpose=True,
                        start=(g % 2 == 0), stop=False,
                    )
                    nc.tensor.matmul(
                        tqk[:, g, 1, :], kl[:, 2 * g : 2 * g + 2, :], idf[:],
                        is_transpose=True,
                        start=False, stop=(g % 2 == 1 or g == nq - 1),
                    )
                qt = mid.tile([128, nq, 128], m1_dt, tag="qt")
                kt = mid.tile([128, nq, 128], m1_dt, tag="kt")
                nc.vector.tensor_copy(qt[:], tqk[:, :, 0, :])
                nc.scalar.copy(kt[:], tqk[:, :, 1, :])

                # ---- M1: S = q @ k^T  (lhsT=qT, rhs=kT) ----
                # pair parity (rh) picks the PSUM bank: concurrent row-groups
                # must not write the same bank+partitions.
                s_a = ps_sa.tile([128, nq, T], F32, tag="s_a")
                s_b = ps_sb.tile([128, nq, T], F32, tag="s_b")
                for t in range(sbt):
                    g, rh, fh = t // 4, (t % 4) // 2, t % 2
                    dst = s_a if rh == 0 else s_b
                    nc.tensor.matmul(
                        dst[fh * 64 : fh * 64 + 64, g, :],
                        qt[rh * 64 : rh * 64 + 64, g, fh * 64 : fh * 64 + 64],
                        kt[rh * 64 : rh * 64 + 64, g, fh * 64 : fh * 64 + 64],
                        start=True, stop=True,
                        tile_position=(rh * 64, fh * 64),
                        skip_group_check=True,
                    )

                # ---- exp + row sums ----
                e_sb = mid.tile([128, sbp, T], m23_dt, tag="e_sb")
                r_sb = mid.tile([128, sbp], F32, tag="r_sb")
                for pb in range(sbp):
                    src = s_a if pb % 2 == 0 else s_b
                    nc.scalar.activation(
                        e_sb[:, pb, :], src[:, pb // 2, :],
                        mybir.ActivationFunctionType.Exp,
                        accum_out=r_sb[:, pb : pb + 1],
                    )

                # ---- transpose E quads on PE (bf16) ----
                te = ps_te.tile([128, nq, 128], m23_dt, tag="te")
                for g in range(nq):
                    nc.tensor.matmul(
                        te[:, g, :], e_sb[:, 2 * g : 2 * g + 2, :], idbf[:],
                        is_transpose=True,
                        start=(g == 0), stop=(g == nq - 1),
                    )
                et = mid.tile([128, nq, 128], m23_dt, tag="et")
                nc.vector.tensor_copy(et[:], te[:])

                # ---- M2: U^T = mm(lhsT=v, rhs=E^T) ----
                u_a = ps_ua.tile([128, nq, T], F32, tag="u_a")
                u_b = ps_ub.tile([128, nq, T], F32, tag="u_b")
                for t in range(sbt):
                    g, rh, fh = t // 4, (t % 4) // 2, t % 2
                    dst = u_a if rh == 0 else u_b
                    nc.tensor.matmul(
                        dst[fh * 64 : fh * 64 + 64, g, :],
                        vl16[rh * 64 : rh * 64 + 64, g, t % 2, :],
                        et[rh * 64 : rh * 64 + 64, g, fh * 64 : fh * 64 + 64],
                        start=True, stop=True,
                        tile_position=(rh * 64, fh * 64),
                        skip_group_check=True,
                    )
                ut = mid.tile([128, sbp, T], m23_dt, tag="ut")
                nc.vector.tensor_copy(ut[:, 0::2, :], u_a[:])
                nc.vector.tensor_copy(ut[:, 1::2, :], u_b[:])

                # ---- M3: P = mm(lhsT=U^T, rhs=proj) ----
                # rows == cols == fh: concurrent row-groups write disjoint
                # partitions, single bank is safe.
                p_ps = ps_p.tile([128, sbp, T], F32, tag="p_ps")
                for t in range(sbt):
                    pb, fh = t // 2, t % 2
                    nc.tensor.matmul(
                        p_ps[fh * 64 : fh * 64 + 64, pb, :],
                        ut[fh * 64 : fh * 64 + 64, pb, :],
                        pl16[fh * 64 : fh * 64 + 64, pb, :],
                        start=True, stop=True,
                        tile_position=(fh * 64, fh * 64),
                        skip_group_check=True,
                    )

                # ---- normalize, add residual (DMA accumulate), store ----
                rinv = mid.tile([128, sbp], F32, tag="rinv")
                nc.vector.reciprocal(rinv[:], r_sb[:])
                o_sb = outp.tile([128, sbp, T], F32, tag="o_sb")
                for pb in range(sbp):
                    nc.scalar.activation(
                        o_sb[:, pb, :], p_ps[:, pb, :],
                        mybir.ActivationFunctionType.Copy,
                        scale=rinv[:, pb : pb + 1],
                    )
                nc.gpsimd.dma_start(out=o_sb[:], in_=rv[s],
                                    accum_op=mybir.AluOpType.add)
                nc.scalar.dma_start(out=ov[s], in_=o_sb[:])

    nc.compile()
    return nc


_BUILT = {}


def _get_built(nt=NT_CORE, sbp=8):
    key = (nt, sbp)
    if key not in _BUILT:
        _BUILT[key] = build(nt, sbp)
    return _BUILT[key]


def kernel(q, k, v, proj, residual):
    from concourse.bass_utils import run_bass_kernel_spmd

    q, k, v, proj, residual = (
        np.ascontiguousarray(np.asarray(x, dtype=np.float32))
        for x in (q, k, v, proj, residual)
    )
    nc = _get_built()
    nt = NT_CORE
    in_maps = []
    for c in range(N_CORES):
        sl = slice(c * nt, (c + 1) * nt)
        in_maps.append(
            {"q": q[sl], "k": k[sl], "v": v[sl], "proj": proj[sl],
             "residual": residual[sl]}
        )
    res = run_bass_kernel_spmd(nc, in_maps, list(range(N_CORES)))
    return np.concatenate([res.results[c]["out"] for c in range(N_CORES)], axis=0)


# revision 11
# speedup vs baseline: 1.8590x; 1.8590x over previous
"""Trainium2 Bass kernel for nn_AttentionBlock64: batch of 8192 independent
64x64 attention tiles, data-parallel across 8 NeuronCores.

out[b] = (softmax(q[b] @ k[b]^T) @ v[b]) @ proj[b] + residual[b]

Math restructure (per tile, all accumulation in fp32 PSUM):
  S   = q @ k^T                    (fp16 inputs, PE)
  E   = exp(S)                     (ACT, fp32 in, bf16 out, accum_out -> rowsum r)
  E^T = transpose(E)               (PE transpose)
  U^T = mm(lhsT=v,   rhs=E^T)      (bf16)
  P   = mm(lhsT=U^T, rhs=proj)     (fp32 psum)
  out = P * (1/r) + residual       (ACT copy-scale; residual added by DMA accum)
The softmax denominator is deferred: row scaling commutes with the
right-multiplications by v and proj.

Layout: tiles processed in pairs stacked on partition halves; quads (4 tiles)
share one 128x128 PE transpose; matmuls are dispatched to distinct 64x64
array quadrants via tile_position so they run concurrently. Concurrent
row-groups must write distinct PSUM banks or distinct partitions (same-bank
same-partition concurrent PE writes wedge the device) - hence the A/B bank
split by pair parity for M1/M2.
"""

import numpy as np

import concourse.bass as bass
import concourse.bacc as bacc
import concourse.mybir as mybir
from concourse import tile

F32 = mybir.dt.float32
F16 = mybir.dt.float16
BF16 = mybir.dt.bfloat16

T = 64          # tile dim
N_CORES = 8
B_FULL = 8192
NT_CORE = B_FULL // N_CORES   # tiles per core


def build(nt=NT_CORE, sbp=8, m1_dt=F16, m23_dt=BF16, reps=1):
    """Build the SPMD single-core program processing `nt` tiles.

    sbp: pairs per superblock (superblock = 2*sbp tiles).
    reps: repeat the whole computation (for timing-by-slope benchmarks).
    """
    sbt = 2 * sbp            # tiles per superblock
    nq = sbp // 2            # quads per superblock
    assert nt % sbt == 0
    nsb = nt // sbt

    nc = bacc.Bacc("TRN2", target_bir_lowering=False, debug=False)

    q_d = nc.dram_tensor("q", [nt, T, T], F32, kind="ExternalInput").ap()
    k_d = nc.dram_tensor("k", [nt, T, T], F32, kind="ExternalInput").ap()
    v_d = nc.dram_tensor("v", [nt, T, T], F32, kind="ExternalInput").ap()
    p_d = nc.dram_tensor("proj", [nt, T, T], F32, kind="ExternalInput").ap()
    r_d = nc.dram_tensor("residual", [nt, T, T], F32, kind="ExternalInput").ap()
    o_d = nc.dram_tensor("out", [nt, T, T], F32, kind="ExternalOutput").ap()

    # pair view: partition = (h i) across the 2 tiles of a pair
    pairv = lambda ap: ap.rearrange("(ns pb h) i j -> ns (h i) pb j", pb=sbp, h=2)
    qv, kv, pv, rv, ov = pairv(q_d), pairv(k_d), pairv(p_d), pairv(r_d), pairv(o_d)
    # v block view: tile (4g+tl) with i on partitions (half chosen by slicing tl)
    vv = v_d.rearrange("(ns g tl) i j -> ns i g tl j", g=nq, tl=4)

    with tile.TileContext(nc) as tc:
        with (
            tc.tile_pool(name="consts", bufs=1) as consts,
            tc.tile_pool(name="qkin", bufs=3) as qkin,
            tc.tile_pool(name="vpin", bufs=3) as vpin,
            tc.tile_pool(name="vp16", bufs=2) as vp16,
            tc.tile_pool(name="mid", bufs=2) as mid,
            tc.tile_pool(name="outp", bufs=3) as outp,
            tc.tile_pool(name="ps_tqk", bufs=1, space="PSUM") as ps_tqk,
            tc.tile_pool(name="ps_sa", bufs=1, space="PSUM") as ps_sa,
            tc.tile_pool(name="ps_sb", bufs=1, space="PSUM") as ps_sb,
            tc.tile_pool(name="ps_te", bufs=1, space="PSUM") as ps_te,
            tc.tile_pool(name="ps_ua", bufs=1, space="PSUM") as ps_ua,
            tc.tile_pool(name="ps_ub", bufs=1, space="PSUM") as ps_ub,
            tc.tile_pool(name="ps_p", bufs=1, space="PSUM") as ps_p,
        ):
            # --- identity matrices for PE transposes ---
            idf = consts.tile([128, 128], F32, tag="idf")
            idbf = consts.tile([128, 128], m23_dt, tag="idbf")
            onesf = consts.tile([128, 128], F32, tag="onesf")
            onesbf = consts.tile([128, 128], m23_dt, tag="onesbf")
            nc.gpsimd.memset(onesf[:], 1.0)
            nc.gpsimd.memset(onesbf[:], 1.0)
            for src, dst in ((onesf, idf), (onesbf, idbf)):
                nc.gpsimd.affine_select(
                    out=dst[:], in_=src[:], pattern=[[-1, 128]],
                    compare_op=mybir.AluOpType.is_equal, fill=0.0,
                    base=0, channel_multiplier=1,
                )

            for s_rep in range(reps * nsb):
                s = s_rep % nsb
                # ---- loads (HWDGE, f32) ----
                ql = qkin.tile([128, sbp, T], F32, tag="ql")
                kl = qkin.tile([128, sbp, T], F32, tag="kl")
                vl = vpin.tile([128, nq, 2, T], F32, tag="vl")
                pl = vpin.tile([128, sbp, T], F32, tag="pl")
                nc.sync.dma_start(out=ql[:], in_=qv[s])
                nc.sync.dma_start(out=kl[:], in_=kv[s])
                for tl in range(4):
                    half = slice(0, 64) if tl < 2 else slice(64, 128)
                    nc.sync.dma_start(out=vl[half, :, tl % 2, :],
                                      in_=vv[s, :, :, tl, :])
                nc.sync.dma_start(out=pl[:], in_=pv[s])

                # ---- cast v/proj to bf16 on gpsimd (otherwise idle) ----
                vl16 = vp16.tile([128, nq, 2, T], m23_dt, tag="vl16")
                pl16 = vp16.tile([128, sbp, T], m23_dt, tag="pl16")
                nc.gpsimd.tensor_copy(vl16[:], vl[:])
                nc.gpsimd.tensor_copy(pl16[:], pl[:])

                # ---- transpose q,k quads on PE (f32) ----
                tqk = ps_tqk.tile([128, nq, 2, 128], F32, tag="tqk")
                # f32 quads: 1KB per quad-slot, zero region = 2KB -> flags
                # restart per pair of quads
                for g in range(nq):
                    nc.tensor.matmul(
                        tqk[:, g, 0, :], ql[:, 2 * g : 2 * g + 2, :], idf[:],
                        is_transpose=True,
                        start=(g % 2 == 0), stop=False,
                    )
                    nc.tensor.matmul(
                        tqk[:, g, 1, :], kl[:, 2 * g : 2 * g + 2, :], idf[:],
                        is_transpose=True,
                        start=False, stop=(g % 2 == 1 or g == nq - 1),
                    )
                qt = mid.tile([128, nq, 128], m1_dt, tag="qt")
                kt = mid.tile([128, nq, 128], m1_dt, tag="kt")
                nc.vector.tensor_copy(qt[:], tqk[:, :, 0, :])
                nc.scalar.copy(kt[:], tqk[:, :, 1, :])

                # ---- M1: S = q @ k^T  (lhsT=qT, rhs=kT) ----
                # pair parity (rh) picks the PSUM bank: concurrent row-groups
                # must not write the same bank+partitions.
                s_a = ps_sa.tile([128, nq, T], F32, tag="s_a")
                s_b = ps_sb.tile([128, nq, T], F32, tag="s_b")
                for t in range(sbt):
                    g, rh, fh = t // 4, (t % 4) // 2, t % 2
                    dst = s_a if rh == 0 else s_b
                    nc.tensor.matmul(
                        dst[fh * 64 : fh * 64 + 64, g, :],
                        qt[rh * 64 : rh * 64 + 64, g, fh * 64 : fh * 64 + 64],
                        kt[rh * 64 : rh * 64 + 64, g, fh * 64 : fh * 64 + 64],
                        start=True, stop=True,
                        tile_position=(rh * 64, fh * 64),
                        skip_group_check=True,
                    )

                # ---- exp (batched per bank) + row sums on DVE ----
                e_sb = mid.tile([128, sbp, T], m23_dt, tag="e_sb")
                r_sb = mid.tile([128, sbp], F32, tag="r_sb")
                nc.scalar.activation(
                    e_sb[:, 0::2, :], s_a[:],
                    mybir.ActivationFunctionType.Exp,
                )
                nc.scalar.activation(
                    e_sb[:, 1::2, :], s_b[:],
                    mybir.ActivationFunctionType.Exp,
                )
                nc.vector.reduce_sum(out=r_sb[:], in_=e_sb[:],
                                     axis=mybir.AxisListType.X)

                # ---- transpose E quads on PE (bf16) ----
                te = ps_te.tile([128, nq, 128], m23_dt, tag="te")
                for g in range(nq):
                    nc.tensor.matmul(
                        te[:, g, :], e_sb[:, 2 * g : 2 * g + 2, :], idbf[:],
                        is_transpose=True,
                        start=(g == 0), stop=(g == nq - 1),
                    )
                et = mid.tile([128, nq, 128], m23_dt, tag="et")
                nc.vector.tensor_copy(et[:], te[:])

                # ---- M2: U^T = mm(lhsT=v, rhs=E^T) ----
                u_a = ps_ua.tile([128, nq, T], F32, tag="u_a")
                u_b = ps_ub.tile([128, nq, T], F32, tag="u_b")
                for t in range(sbt):
                    g, rh, fh = t // 4, (t % 4) // 2, t % 2
                    dst = u_a if rh == 0 else u_b
                    nc.tensor.matmul(
                        dst[fh * 64 : fh * 64 + 64, g, :],
                        vl16[rh * 64 : rh * 64 + 64, g, t % 2, :],
                        et[rh * 64 : rh * 64 + 64, g, fh * 64 : fh * 64 + 64],
                        start=True, stop=True,
                        tile_position=(rh * 64, fh * 64),
                        skip_group_check=True,
                    )
                ut = mid.tile([128, sbp, T], m23_dt, tag="ut")
                nc.vector.tensor_copy(ut[:, 0::2, :], u_a[:])
                nc.vector.tensor_copy(ut[:, 1::2, :], u_b[:])

                # ---- M3: P = mm(lhsT=U^T, rhs=proj) ----
                # rows == cols == fh: concurrent row-groups write disjoint
                # partitions, single bank is safe.
                p_ps = ps_p.tile([128, sbp, T], F32, tag="p_ps")
                for t in range(sbt):
                    pb, fh = t // 2, t % 2
                    nc.tensor.matmul(
                        p_ps[fh * 64 : fh * 64 + 64, pb, :],
                        ut[fh * 64 : fh * 64 + 64, pb, :],
                        pl16[fh * 64 : fh * 64 + 64, pb, :],
                        start=True, stop=True,
                        tile_position=(fh * 64, fh * 64),
                        skip_group_check=True,
                    )

                # ---- normalize, add residual (DMA accumulate), store ----
                rinv = mid.tile([128, sbp], F32, tag="rinv")
                nc.vector.reciprocal(rinv[:], r_sb[:])
                o_sb = outp.tile([128, sbp, T], F32, tag="o_sb")
                for pb in range(sbp):
                    nc.scalar.activation(
                        o_sb[:, pb, :], p_ps[:, pb, :],
                        mybir.ActivationFunctionType.Copy,
                        scale=rinv[:, pb : pb + 1],
                    )
                nc.gpsimd.dma_start(out=o_sb[:], in_=rv[s],
                                    accum_op=mybir.AluOpType.add)
                nc.scalar.dma_start(out=ov[s], in_=o_sb[:])

    nc.compile()
    return nc


_BUILT = {}


def _get_built(nt=NT_CORE, sbp=8):
    key = (nt, sbp)
    if key not in _BUILT:
        _BUILT[key] = build(nt, sbp)
    return _BUILT[key]


def kernel(q, k, v, proj, residual):
    from concourse.bass_utils import run_bass_kernel_spmd

    q, k, v, proj, residual = (
        np.ascontiguousarray(np.asarray(x, dtype=np.float32))
        for x in (q, k, v, proj, residual)
    )
    nc = _get_built()
    nt = NT_CORE
    in_maps = []
    for c in range(N_CORES):
        sl = slice(c * nt, (c + 1) * nt)
        in_maps.append(
            {"q": q[sl], "k": k[sl], "v": v[sl], "proj": proj[sl],
             "residual": residual[sl]}
        )
    res = run_bass_kernel_spmd(nc, in_maps, list(range(N_CORES)))
    return np.concatenate([res.results[c]["out"] for c in range(N_CORES)], axis=0)
